# revision 65
# baseline (speedup 1.0000x reference)
"""Bahdanau attention Trainium2 kernel.

Shapes (full problem): query [32,1,2048], keys [32,2048,1024],
Wa [2048,1024], Ua [1024,1024], Va [1024,1]; outputs
context [32,1,1024], weights [32,1,2048].

Strategy: data-parallel over batch B=32 across 8 cores (4 batches/core),
weights replicated. Per core, single pass over keys:
  - keys chunk [512s, 1024d] loaded naturally, PE-transposed to
    keysT [d,s] tiles (fp32r, 1.5 cyc/row)
  - GEMM k_projT[h,s] += Ua[d,h].T-stationary @ keysT[d,s] (fp32r, N=512)
  - ACT tanh with per-partition bias = (q + Wa_b + Ua_b)[h]  -> e[h,s]
  - PE matvec scores[s] = va.T @ e  (accumulated over 8 h-tiles)
  - exp(scores) without max subtraction (|scores| <= ||Va||_1 ~ 26, exp
    safe in fp32; softmax is shift/Va_b invariant so Va_b is dropped)
  - PE matvec context += expw.T-stationary @ keys_chunk (keys still in
    SBUF; normalized by 1/sum(exp) at the end)
"""

import sys

if "/opt/trn_rl_repo" not in sys.path:
    sys.path.insert(0, "/opt/trn_rl_repo")

from contextlib import ExitStack

import numpy as np

import concourse.bass as bass
import concourse.tile as tile
from concourse import bacc, mybir
from concourse.masks import make_identity
from concourse.tile_rust import add_dep_helper

B, S, H, L = 32, 2048, 1024, 2
NCORES = 8
BL = B // NCORES          # 4 batches per core
P = 128
CHUNK = 512               # s columns per processing chunk
TPB = S // CHUNK          # 4 chunks per batch
NCHUNK = BL * TPB         # 16 chunks per core
HT = H // P               # 8 h tiles
DT = H // P               # 8 d tiles (Ua contraction)
D2T = (H * L) // P        # 16 d2 tiles (Wa contraction)
SB = CHUNK // P           # 4 s-subtiles per chunk

F32 = mybir.dt.float32
F32R = mybir.dt.float32r
BF16 = mybir.dt.bfloat16
TANH = mybir.ActivationFunctionType.Tanh
EXP = mybir.ActivationFunctionType.Exp


def build_program() -> bass.Bass:
    nc = bacc.Bacc()

    query_d = nc.declare_dram_parameter("query", [BL, H * L], F32, isOutput=False)
    keys_d = nc.declare_dram_parameter("keys", [BL, S, H], F32, isOutput=False)
    wa_d = nc.declare_dram_parameter("wa", [H * L, H], F32, isOutput=False)
    wab_d = nc.declare_dram_parameter("wab", [H], F32, isOutput=False)
    ua_d = nc.declare_dram_parameter("ua", [H, H], F32, isOutput=False)
    uab_d = nc.declare_dram_parameter("uab", [H], F32, isOutput=False)
    va_d = nc.declare_dram_parameter("va", [H, 1], F32, isOutput=False)
    ctx_d = nc.declare_dram_parameter("context", [BL, H], F32, isOutput=True)
    wout_d = nc.declare_dram_parameter("weights", [BL, S], F32, isOutput=True)

    with tile.TileContext(nc) as tc, ExitStack() as ctx:
        _body(ctx, tc, query_d, keys_d, wa_d, wab_d, ua_d, uab_d, va_d, ctx_d, wout_d)
    nc.finalize()
    return nc


def _body(ctx, tc, query_d, keys_d, wa_d, wab_d, ua_d, uab_d, va_d, ctx_d, wout_d):
    nc = tc.nc

    const = ctx.enter_context(tc.tile_pool(name="const", bufs=1))
    wa_pool = ctx.enter_context(tc.tile_pool(name="wa", bufs=8))
    keys_pool = ctx.enter_context(tc.tile_pool(name="keys", bufs=3))
    kt_pool = ctx.enter_context(tc.tile_pool(name="kt", bufs=16))
    e_pool = ctx.enter_context(tc.tile_pool(name="e", bufs=8))
    work = ctx.enter_context(tc.tile_pool(name="work", bufs=3))

    tp_ps = ctx.enter_context(tc.tile_pool(name="tp_ps", bufs=3, space="PSUM"))
    g_ps = ctx.enter_context(tc.tile_pool(name="g_ps", bufs=2, space="PSUM"))
    sc_ps = ctx.enter_context(tc.tile_pool(name="sc_ps", bufs=1, space="PSUM"))
    cx_ps = ctx.enter_context(tc.tile_pool(name="cx_ps", bufs=2, space="PSUM"))

    # ---- constants / replicated weights ----
    # Everything consumed by an fp32r matmul must be *produced* as fp32r
    # (BIR verifier rule), so feeding DMAs/copies are bitcast on both sides.
    identity = const.tile([P, P], F32)
    make_identity(nc, identity)
    identity_r = const.tile([P, P], F32)
    nc.vector.tensor_copy(
        out=identity_r.bitcast(F32R), in_=identity.bitcast(F32R)
    )


    # keys chunk 0/1 DMAs are emitted before the Ua load (see below, after
    # the emitters are defined) so the PE-critical path starts first.
    ua_sb = const.tile([P, DT, H], F32)  # ua_sb[p, di, h] = Ua[di*128+p, h]

    def emit_ua_load(after_dma=None):
        dma = None
        for di in range(DT):
            dma = nc.sync.dma_start(
                out=ua_sb[:, di, :].bitcast(F32R),
                in_=ua_d[di * P : (di + 1) * P, :].bitcast(F32R),
            )
            if di == 0 and after_dma is not None:
                # serialize the bulk preload behind the PE-critical chunk-0
                # keys DMA (SP queue is in-order, so one edge gates the rest)
                add_dep_helper(
                    dma.ins, after_dma.ins, reason="keys chunk 0 lands first"
                )
        return dma

    va_nat = const.tile([1, H], F32)
    va_sb = const.tile([P, HT], F32)  # va_sb[p, hi] = Va[hi*128+p, 0]
    wab_bc = const.tile([BL, H], F32)
    uab_bc = const.tile([BL, H], F32)
    query_sb = const.tile([BL, H * L], F32)

    def emit_misc_consts():
        # va as [h, hi] columns: natural [1, H] load + on-chip PE transpose
        nc.sync.dma_start(out=va_nat, in_=va_d[:, :].rearrange("h one -> one h"))
        va_ps = tp_ps.tile([P, CHUNK], F32, tag="tp", name="va_ps")
        for hi in range(HT):
            nc.tensor.transpose(
                va_ps[:, hi : hi + 1],
                va_nat[:, hi * P : (hi + 1) * P],
                identity[:1, :1],
            )
        nc.vector.tensor_copy(out=va_sb.bitcast(F32R), in_=va_ps[:, :HT].bitcast(F32R))

        # (Wa_b + Ua_b) broadcast to the BL query rows
        def _bcast(ap_1d):
            return bass.AP(
                tensor=ap_1d.tensor, offset=ap_1d.offset, ap=[[0, BL]] + list(ap_1d.ap)
            )

        nc.gpsimd.dma_start(out=wab_bc, in_=_bcast(wab_d[:]))
        nc.gpsimd.dma_start(out=uab_bc, in_=_bcast(uab_d[:]))
        nc.sync.dma_start(out=query_sb, in_=query_d[:, :])

    # ---- chunk-stage emitters (software pipeline over the key chunks) ----
    RP = 32  # batch b lives on partition 32*b (engines start on quads)
    st = {}  # per-chunk state

    def emit_load(c):
        # 4 parallel DMAs (one per s-subtile) — a single big call serializes
        # into latency-dominated sub-transfers on one queue
        b, cc = divmod(c, TPB)
        kchunk = keys_pool.tile([P, SB, H], F32, tag="keys", name=f"kchunk{c}")
        dma = None
        for t in range(SB):
            dma = nc.sync.dma_start(
                out=kchunk[:, t, :].bitcast(F32R),
                in_=keys_d[b, cc * CHUNK + t * P : cc * CHUNK + (t + 1) * P, :]
                .bitcast(F32R),
            )
        st[c] = {"kchunk": kchunk}
        return dma

    def emit_transpose_group(c, di):
        # kts[di][p_d, s] = keys[s, di*128+p_d]
        kchunk = st[c]["kchunk"]
        ktp = tp_ps.tile([P, CHUNK], F32, tag="tp", name=f"ktp{c}_{di}")
        for t in range(SB):
            nc.tensor.transpose(
                ktp[:, t * P : (t + 1) * P].bitcast(F32R),
                kchunk[:, t, di * P : (di + 1) * P].bitcast(F32R),
                identity_r.bitcast(F32R),
            )
        kt_sb = kt_pool.tile([P, CHUNK], F32, tag="kt", name=f"kt{c}_{di}")
        # alternate the PSUM->SBUF drain between DVE and ACT so the
        # copy throughput does not pace the PE transposes
        if di % 2 == 0:
            nc.vector.tensor_copy(out=kt_sb.bitcast(F32R), in_=ktp.bitcast(F32R))
        else:
            nc.scalar.copy(out=kt_sb.bitcast(F32R), in_=ktp.bitcast(F32R))
        st[c].setdefault("kts", []).append(kt_sb)

    def emit_transpose(c):
        for di in range(DT):
            emit_transpose_group(c, di)

    def emit_tanh(c, hi, gp):
        b = c // TPB
        es = st[c].setdefault("es", [None] * HT)
        e_sb = e_pool.tile([P, CHUNK], F32, tag="e", name=f"e{c}_{hi}")
        nc.scalar.activation(
            out=e_sb.bitcast(F32R),
            in_=gp,
            func=TANH,
            bias=qbT_sb[:, hi, b : b + 1],
            scale=1.0,
        )
        es[hi] = e_sb

    def emit_gemm(c, hi):
        kts = st[c]["kts"]
        gp = g_ps.tile([P, CHUNK], F32, tag="g", name=f"g{c}_{hi}")
        for di in range(DT):
            nc.tensor.matmul(
                gp,
                ua_sb[:, di, hi * P : (hi + 1) * P].bitcast(F32R),
                kts[di].bitcast(F32R),
                start=(di == 0),
                stop=(di == DT - 1),
            )
        return gp

    def emit_gemm_tanh(c, qbT_sb, hs=None):
        for hi in hs if hs is not None else range(HT):
            gp = emit_gemm(c, hi)
            emit_tanh(c, hi, gp)

    def emit_scores(c, scores_sb):
        b, cc = divmod(c, TPB)
        es = st[c]["es"]
        scp = sc_ps.tile([1, CHUNK], F32, tag="sc", name=f"scp{c}")
        for hi in range(HT):
            nc.tensor.matmul(
                scp,
                va_sb[:, hi : hi + 1].bitcast(F32R),
                es[hi].bitcast(F32R),
                start=(hi == 0),
                stop=(hi == HT - 1),
            )
        nc.vector.tensor_copy(
            out=scores_sb[b * RP : b * RP + 1, cc * CHUNK : (cc + 1) * CHUNK],
            in_=scp,
        )
        st[c]["scp"] = scp

    def emit_ctx(c, cxacc_ref, ctx_raw):
        # exp(scores) -> transpose to [s,1] -> context matvec accumulate
        b, cc = divmod(c, TPB)
        scp = st[c]["scp"]
        kchunk = st[c]["kchunk"]
        ew_sb = work.tile([1, CHUNK], F32, tag="ew", name=f"ew{c}")
        nc.scalar.activation(out=ew_sb, in_=scp, func=EXP)
        ewt_ps = tp_ps.tile([P, CHUNK], F32, tag="tp", name=f"ewt{c}")
        for t in range(SB):
            nc.tensor.transpose(
                ewt_ps[:, t : t + 1], ew_sb[:, t * P : (t + 1) * P], identity[:1, :1]
            )
        ewT_sb = work.tile([P, SB], F32, tag="ewT", name=f"ewT{c}")
        nc.vector.tensor_copy(
            out=ewT_sb.bitcast(F32R), in_=ewt_ps[:, :SB].bitcast(F32R)
        )
        if cc == 0:
            cxacc_ref[0] = [
                cx_ps.tile([1, 512], F32, tag="cx", name=f"cxacc{c}_{i}")
                for i in range(2)
            ]
        cxacc = cxacc_ref[0]
        for t in range(SB):
            for half in range(2):
                nc.tensor.matmul(
                    cxacc[half],
                    ewT_sb[:, t : t + 1].bitcast(F32R),
                    kchunk[:, t, half * 512 : (half + 1) * 512].bitcast(F32R),
                    start=(cc == 0 and t == 0),
                    stop=(cc == TPB - 1 and t == SB - 1),
                )
        if cc == TPB - 1:
            for half in range(2):
                nc.vector.tensor_copy(
                    out=ctx_raw[b * RP : b * RP + 1, half * 512 : (half + 1) * 512],
                    in_=cxacc[half],
                )
        del st[c]["scp"], st[c]["kchunk"]

    # prefetch the first chunks and keep PE busy while Wa streams in
    # ---- phase Q pieces: q = query @ Wa (+ biases), laid out as [h, b] ----
    qT_sb = const.tile([P, D2T, BL], F32)  # query transposed: [d2, j, b]
    q_sb = const.tile([BL, H], F32)
    qbT_sb = const.tile([P, HT, BL], F32)  # [h_in_tile, hi, b] tanh bias

    def emit_qT():
        for j in range(D2T):
            tp = tp_ps.tile([P, CHUNK], F32, tag="tp", name=f"qTp{j}")
            nc.tensor.transpose(
                tp[:, :BL], query_sb[:, j * P : (j + 1) * P], identity[:BL, :BL]
            )
            nc.vector.tensor_copy(
                out=qT_sb[:, j, :].bitcast(F32R), in_=tp[:, :BL].bitcast(F32R)
            )

    wa_tiles = {}

    def emit_wa_dmas(half, after_dma):
        sl = slice(half * 512, (half + 1) * 512)
        for j in range(D2T):
            wa_t = wa_pool.tile([P, 512], F32, tag="wa", name=f"wa{half}_{j}")
            dma = nc.sync.dma_start(
                out=wa_t.bitcast(F32R),
                in_=wa_d[j * P : (j + 1) * P, sl].bitcast(F32R),
            )
            if half == 0 and j == 0 and after_dma is not None:
                add_dep_helper(
                    dma.ins, after_dma.ins, reason="wa after ua preload"
                )
            wa_tiles[(half, j)] = wa_t

    def emit_q_mms(half):
        # q for h columns [half*512, (half+1)*512)
        sl = slice(half * 512, (half + 1) * 512)
        q_ps = cx_ps.tile([BL, 512], F32, tag="cx", name=f"q_ps{half}")
        for j in range(D2T):
            nc.tensor.matmul(
                q_ps,
                qT_sb[:, j, :].bitcast(F32R),
                wa_tiles[(half, j)].bitcast(F32R),
                start=(j == 0),
                stop=(j == D2T - 1),
            )
        nc.vector.tensor_add(q_sb[:, sl], q_ps, wab_bc[:, sl])
        nc.vector.tensor_add(q_sb[:, sl], q_sb[:, sl], uab_bc[:, sl])
        for hi in range(half * (HT // 2), (half + 1) * (HT // 2)):
            tp = tp_ps.tile([P, CHUNK], F32, tag="tp", name=f"qbTp{hi}")
            nc.tensor.transpose(
                tp[:, :BL], q_sb[:, hi * P : (hi + 1) * P], identity[:BL, :BL]
            )
            nc.vector.tensor_copy(out=qbT_sb[:, hi, :], in_=tp[:, :BL])

    # ---- startup orchestration ----
    # DMA chain: keys chunk 0 -> Ua -> {keys chunk 1, Wa}.  PE order: T(0),
    # qT, GEMM c0 h0-1, qMM-half0, GEMM h2-3, qMM-half1, GEMM h4-7 — the
    # splits respect the 2 GEMM psum slots (drained by tanh, which needs q).
    dma0 = emit_load(0)
    emit_misc_consts()
    ua_last = emit_ua_load(after_dma=dma0)
    k1_dma = emit_load(1)
    add_dep_helper(k1_dma.ins, ua_last.ins, reason="keys 1 after ua")
    emit_transpose(0)
    emit_qT()
    gp0 = emit_gemm(0, 0)
    gp1 = emit_gemm(0, 1)
    emit_wa_dmas(0, after_dma=ua_last)
    emit_wa_dmas(1, after_dma=None)
    emit_q_mms(0)
    emit_tanh(0, 0, gp0)
    emit_tanh(0, 1, gp1)
    emit_gemm_tanh(0, qbT_sb, hs=[2, 3])
    emit_q_mms(1)
    emit_gemm_tanh(0, qbT_sb, hs=[4, 5, 6, 7])

    # ---- main pass over keys (1-chunk-lag software pipeline) ----
    scores_sb = const.tile([P, S], F32)
    ctx_raw = const.tile([P, H], F32)  # unnormalized context rows
    w_sb = const.tile([P, S], F32)
    z_sb = const.tile([P, 1], F32)
    rz_sb = const.tile([P, 1], F32)
    ctx_sb = const.tile([P, H], F32)

    def emit_batch_tail(b):
        # softmax normalization + outputs for one finished batch; overlaps
        # the next batch's chunks (only the last batch's tail is serial)
        row = slice(b * RP, b * RP + 1)
        nc.scalar.activation(
            out=w_sb[row, :], in_=scores_sb[row, :], func=EXP, accum_out=z_sb[row, :]
        )
        nc.vector.reciprocal(out=rz_sb[row, :], in_=z_sb[row, :])
        nc.vector.tensor_scalar_mul(w_sb[row, :], w_sb[row, :], rz_sb[row, :])
        nc.sync.dma_start(out=wout_d[b : b + 1, :], in_=w_sb[row, :])
        nc.vector.tensor_scalar_mul(ctx_sb[row, :], ctx_raw[row, :], rz_sb[row, :])
        nc.sync.dma_start(out=ctx_d[b : b + 1, :], in_=ctx_sb[row, :])

    cxacc_ref = [None]
    for c in range(NCHUNK):
        # chunk c-1's exp -> ewT -> context ride on PE ahead of GEMM c,
        # so the ACT exp latency is hidden under the previous GEMM.
        if c > 0:
            emit_ctx(c - 1, cxacc_ref, ctx_raw)
            if c % TPB == 0:
                emit_batch_tail(c // TPB - 1)
        if c + 1 < NCHUNK and c + 2 < NCHUNK:
            emit_load(c + 2)
        if c == 0:
            # chunk 0's GEMM was emitted during startup
            emit_transpose(1)
        else:
            # interleave next chunk's transpose groups between GEMM h-groups
            # so their LDWEIGHTS hide under the 512-cycle GEMM matmuls
            for hi in range(HT):
                gp = emit_gemm(c, hi)
                emit_tanh(c, hi, gp)
                if c + 1 < NCHUNK:
                    emit_transpose_group(c + 1, hi)
        emit_scores(c, scores_sb)
    emit_ctx(NCHUNK - 1, cxacc_ref, ctx_raw)
    emit_batch_tail(BL - 1)


_CACHED_NC = None


def _get_nc():
    global _CACHED_NC
    if _CACHED_NC is None:
        _CACHED_NC = build_program()
    return _CACHED_NC


def make_in_maps(query, keys, Wa_w, Wa_b, Ua_w, Ua_b, Va_w):
    """Shard full inputs into 8 per-core input maps (data parallel over B)."""
    query = np.ascontiguousarray(np.asarray(query, dtype=np.float32)).reshape(B, H * L)
    keys = np.ascontiguousarray(np.asarray(keys, dtype=np.float32))
    shared = {
        "wa": np.ascontiguousarray(np.asarray(Wa_w, dtype=np.float32)),
        "wab": np.ascontiguousarray(np.asarray(Wa_b, dtype=np.float32)),
        "ua": np.ascontiguousarray(np.asarray(Ua_w, dtype=np.float32)),
        "uab": np.ascontiguousarray(np.asarray(Ua_b, dtype=np.float32)),
        "va": np.ascontiguousarray(np.asarray(Va_w, dtype=np.float32)),
    }
    in_maps = []
    for c in range(NCORES):
        sl = slice(c * BL, (c + 1) * BL)
        in_maps.append(
            {
                "query": np.ascontiguousarray(query[sl]),
                "keys": np.ascontiguousarray(keys[sl]),
                **shared,
            }
        )
    return in_maps


def assemble_outputs(results):
    ctxs = [np.asarray(r["context"]) for r in results]
    ws = [np.asarray(r["weights"]) for r in results]
    context = np.concatenate(ctxs, axis=0).reshape(B, 1, H)
    weights = np.concatenate(ws, axis=0).reshape(B, 1, S)
    return context.astype(np.float32), weights.astype(np.float32)


def run_on_hw(in_maps, trace=False, tmpdir=None):
    from concourse.bass_utils import run_bass_kernel_spmd

    nc = _get_nc()
    return run_bass_kernel_spmd(
        nc, in_maps, list(range(NCORES)), trace=trace, tmpdir=tmpdir
    )


def kernel(query, keys, Wa_w, Wa_b, Ua_w, Ua_b, Va_w, Va_b=None):
    # Va_b is mathematically irrelevant: softmax(s + c) == softmax(s), and
    # both outputs depend on the scores only through the softmax.
    in_maps = make_in_maps(query, keys, Wa_w, Wa_b, Ua_w, Ua_b, Va_w)
    res = run_on_hw(in_maps, trace=False)
    return assemble_outputs(res.results)


if __name__ == "__main__":
    nc = build_program()
    print("program built ok")


# revision 66
# speedup vs baseline: 1.0231x; 1.0231x over previous
"""Bahdanau attention Trainium2 kernel.

Shapes (full problem): query [32,1,2048], keys [32,2048,1024],
Wa [2048,1024], Ua [1024,1024], Va [1024,1]; outputs
context [32,1,1024], weights [32,1,2048].

Strategy: data-parallel over batch B=32 across 8 cores (4 batches/core),
weights replicated. Per core, single pass over keys:
  - keys chunk [512s, 1024d] loaded naturally, PE-transposed to
    keysT [d,s] tiles (fp32r, 1.5 cyc/row)
  - GEMM k_projT[h,s] += Ua[d,h].T-stationary @ keysT[d,s] (fp32r, N=512)
  - ACT tanh with per-partition bias = (q + Wa_b + Ua_b)[h]  -> e[h,s]
  - PE matvec scores[s] = va.T @ e  (accumulated over 8 h-tiles)
  - exp(scores) without max subtraction (|scores| <= ||Va||_1 ~ 26, exp
    safe in fp32; softmax is shift/Va_b invariant so Va_b is dropped)
  - PE matvec context += expw.T-stationary @ keys_chunk (keys still in
    SBUF; normalized by 1/sum(exp) at the end)
"""

import sys

if "/opt/trn_rl_repo" not in sys.path:
    sys.path.insert(0, "/opt/trn_rl_repo")

from contextlib import ExitStack

import numpy as np

import concourse.bass as bass
import concourse.tile as tile
from concourse import bacc, mybir
from concourse.masks import make_identity
from concourse.tile_rust import add_dep_helper

B, S, H, L = 32, 2048, 1024, 2
NCORES = 8
BL = B // NCORES          # 4 batches per core
P = 128
CHUNK = 512               # s columns per processing chunk
TPB = S // CHUNK          # 4 chunks per batch
NCHUNK = BL * TPB         # 16 chunks per core
HT = H // P               # 8 h tiles
DT = H // P               # 8 d tiles (Ua contraction)
D2T = (H * L) // P        # 16 d2 tiles (Wa contraction)
SB = CHUNK // P           # 4 s-subtiles per chunk

F32 = mybir.dt.float32
F32R = mybir.dt.float32r
BF16 = mybir.dt.bfloat16
TANH = mybir.ActivationFunctionType.Tanh
EXP = mybir.ActivationFunctionType.Exp


def build_program() -> bass.Bass:
    nc = bacc.Bacc()

    query_d = nc.declare_dram_parameter("query", [BL, H * L], F32, isOutput=False)
    keys_d = nc.declare_dram_parameter("keys", [BL, S, H], F32, isOutput=False)
    wa_d = nc.declare_dram_parameter("wa", [H * L, H], F32, isOutput=False)
    wab_d = nc.declare_dram_parameter("wab", [H], F32, isOutput=False)
    ua_d = nc.declare_dram_parameter("ua", [H, H], F32, isOutput=False)
    uab_d = nc.declare_dram_parameter("uab", [H], F32, isOutput=False)
    va_d = nc.declare_dram_parameter("va", [H, 1], F32, isOutput=False)
    ctx_d = nc.declare_dram_parameter("context", [BL, H], F32, isOutput=True)
    wout_d = nc.declare_dram_parameter("weights", [BL, S], F32, isOutput=True)

    with tile.TileContext(nc) as tc, ExitStack() as ctx:
        _body(ctx, tc, query_d, keys_d, wa_d, wab_d, ua_d, uab_d, va_d, ctx_d, wout_d)
    nc.finalize()
    return nc


def _body(ctx, tc, query_d, keys_d, wa_d, wab_d, ua_d, uab_d, va_d, ctx_d, wout_d):
    nc = tc.nc

    const = ctx.enter_context(tc.tile_pool(name="const", bufs=1))
    wa_pool = ctx.enter_context(tc.tile_pool(name="wa", bufs=8))
    keys_pool = ctx.enter_context(tc.tile_pool(name="keys", bufs=3))
    kt_pool = ctx.enter_context(tc.tile_pool(name="kt", bufs=10))
    e_pool = ctx.enter_context(tc.tile_pool(name="e", bufs=10))
    work = ctx.enter_context(tc.tile_pool(name="work", bufs=3))

    tp_ps = ctx.enter_context(tc.tile_pool(name="tp_ps", bufs=3, space="PSUM"))
    g_ps = ctx.enter_context(tc.tile_pool(name="g_ps", bufs=2, space="PSUM"))
    sc_ps = ctx.enter_context(tc.tile_pool(name="sc_ps", bufs=1, space="PSUM"))
    cx_ps = ctx.enter_context(tc.tile_pool(name="cx_ps", bufs=2, space="PSUM"))

    # ---- constants / replicated weights ----
    # Everything consumed by an fp32r matmul must be *produced* as fp32r
    # (BIR verifier rule), so feeding DMAs/copies are bitcast on both sides.
    identity = const.tile([P, P], F32)
    make_identity(nc, identity)
    identity_r = const.tile([P, P], F32)
    nc.vector.tensor_copy(
        out=identity_r.bitcast(F32R), in_=identity.bitcast(F32R)
    )


    # keys chunk 0/1 DMAs are emitted before the Ua load (see below, after
    # the emitters are defined) so the PE-critical path starts first.
    ua_sb = const.tile([P, DT, H], F32)  # ua_sb[p, di, h] = Ua[di*128+p, h]

    def emit_ua_load(after_dma=None):
        dma = None
        for di in range(DT):
            dma = nc.sync.dma_start(
                out=ua_sb[:, di, :].bitcast(F32R),
                in_=ua_d[di * P : (di + 1) * P, :].bitcast(F32R),
            )
            if di == 0 and after_dma is not None:
                # serialize the bulk preload behind the PE-critical chunk-0
                # keys DMA (SP queue is in-order, so one edge gates the rest)
                add_dep_helper(
                    dma.ins, after_dma.ins, reason="keys chunk 0 lands first"
                )
        return dma

    va_nat = const.tile([1, H], F32)
    va_sb = const.tile([P, HT], F32)  # va_sb[p, hi] = Va[hi*128+p, 0]
    wab_bc = const.tile([BL, H], F32)
    uab_bc = const.tile([BL, H], F32)
    query_sb = const.tile([BL, H * L], F32)

    def emit_misc_consts():
        # va as [h, hi] columns: natural [1, H] load + on-chip PE transpose
        nc.sync.dma_start(out=va_nat, in_=va_d[:, :].rearrange("h one -> one h"))
        va_ps = tp_ps.tile([P, CHUNK], F32, tag="tp", name="va_ps")
        for hi in range(HT):
            nc.tensor.transpose(
                va_ps[:, hi : hi + 1],
                va_nat[:, hi * P : (hi + 1) * P],
                identity[:1, :1],
            )
        nc.vector.tensor_copy(out=va_sb.bitcast(F32R), in_=va_ps[:, :HT].bitcast(F32R))

        # (Wa_b + Ua_b) broadcast to the BL query rows
        def _bcast(ap_1d):
            return bass.AP(
                tensor=ap_1d.tensor, offset=ap_1d.offset, ap=[[0, BL]] + list(ap_1d.ap)
            )

        nc.gpsimd.dma_start(out=wab_bc, in_=_bcast(wab_d[:]))
        nc.gpsimd.dma_start(out=uab_bc, in_=_bcast(uab_d[:]))
        nc.sync.dma_start(out=query_sb, in_=query_d[:, :])

    # ---- chunk-stage emitters (software pipeline over the key chunks) ----
    RP = 32  # batch b lives on partition 32*b (engines start on quads)
    st = {}  # per-chunk state

    def emit_load(c):
        # 4 parallel DMAs (one per s-subtile) — a single big call serializes
        # into latency-dominated sub-transfers on one queue
        b, cc = divmod(c, TPB)
        kchunk = keys_pool.tile([P, SB, H], F32, tag="keys", name=f"kchunk{c}")
        dma = None
        for t in range(SB):
            dma = nc.sync.dma_start(
                out=kchunk[:, t, :].bitcast(F32R),
                in_=keys_d[b, cc * CHUNK + t * P : cc * CHUNK + (t + 1) * P, :]
                .bitcast(F32R),
            )
        st[c] = {"kchunk": kchunk}
        return dma

    def emit_transpose(c):
        # kts[di][p_d, s] = keys[s, di*128+p_d]
        kchunk = st[c]["kchunk"]
        kts = []
        for di in range(DT):
            ktp = tp_ps.tile([P, CHUNK], F32, tag="tp", name=f"ktp{c}_{di}")
            for t in range(SB):
                nc.tensor.transpose(
                    ktp[:, t * P : (t + 1) * P].bitcast(F32R),
                    kchunk[:, t, di * P : (di + 1) * P].bitcast(F32R),
                    identity_r.bitcast(F32R),
                )
            kt_sb = kt_pool.tile([P, CHUNK], F32, tag="kt", name=f"kt{c}_{di}")
            # alternate the PSUM->SBUF drain between DVE and ACT so the
            # copy throughput does not pace the PE transposes
            if di % 2 == 0:
                nc.vector.tensor_copy(out=kt_sb.bitcast(F32R), in_=ktp.bitcast(F32R))
            else:
                nc.scalar.copy(out=kt_sb.bitcast(F32R), in_=ktp.bitcast(F32R))
            kts.append(kt_sb)
        st[c]["kts"] = kts

    def emit_tanh(c, hi, gp):
        b = c // TPB
        es = st[c].setdefault("es", [None] * HT)
        e_sb = e_pool.tile([P, CHUNK], F32, tag="e", name=f"e{c}_{hi}")
        nc.scalar.activation(
            out=e_sb.bitcast(F32R),
            in_=gp,
            func=TANH,
            bias=qbT_sb[:, hi, b : b + 1],
            scale=1.0,
        )
        es[hi] = e_sb

    def emit_gemm(c, hi):
        kts = st[c]["kts"]
        gp = g_ps.tile([P, CHUNK], F32, tag="g", name=f"g{c}_{hi}")
        for di in range(DT):
            nc.tensor.matmul(
                gp,
                ua_sb[:, di, hi * P : (hi + 1) * P].bitcast(F32R),
                kts[di].bitcast(F32R),
                start=(di == 0),
                stop=(di == DT - 1),
            )
        return gp

    def emit_gemm_tanh(c, qbT_sb, hs=None):
        for hi in hs if hs is not None else range(HT):
            gp = emit_gemm(c, hi)
            emit_tanh(c, hi, gp)

    def emit_scores(c, scores_sb):
        b, cc = divmod(c, TPB)
        es = st[c]["es"]
        scp = sc_ps.tile([1, CHUNK], F32, tag="sc", name=f"scp{c}")
        for hi in range(HT):
            nc.tensor.matmul(
                scp,
                va_sb[:, hi : hi + 1].bitcast(F32R),
                es[hi].bitcast(F32R),
                start=(hi == 0),
                stop=(hi == HT - 1),
            )
        nc.vector.tensor_copy(
            out=scores_sb[b * RP : b * RP + 1, cc * CHUNK : (cc + 1) * CHUNK],
            in_=scp,
        )
        st[c]["scp"] = scp

    def emit_ctx(c, cxacc_ref, ctx_raw):
        # exp(scores) -> transpose to [s,1] -> context matvec accumulate
        b, cc = divmod(c, TPB)
        scp = st[c]["scp"]
        kchunk = st[c]["kchunk"]
        ew_sb = work.tile([1, CHUNK], F32, tag="ew", name=f"ew{c}")
        nc.scalar.activation(out=ew_sb, in_=scp, func=EXP)
        ewt_ps = tp_ps.tile([P, CHUNK], F32, tag="tp", name=f"ewt{c}")
        for t in range(SB):
            nc.tensor.transpose(
                ewt_ps[:, t : t + 1], ew_sb[:, t * P : (t + 1) * P], identity[:1, :1]
            )
        ewT_sb = work.tile([P, SB], F32, tag="ewT", name=f"ewT{c}")
        nc.vector.tensor_copy(
            out=ewT_sb.bitcast(F32R), in_=ewt_ps[:, :SB].bitcast(F32R)
        )
        if cc == 0:
            cxacc_ref[0] = [
                cx_ps.tile([1, 512], F32, tag="cx", name=f"cxacc{c}_{i}")
                for i in range(2)
            ]
        cxacc = cxacc_ref[0]
        for t in range(SB):
            for half in range(2):
                nc.tensor.matmul(
                    cxacc[half],
                    ewT_sb[:, t : t + 1].bitcast(F32R),
                    kchunk[:, t, half * 512 : (half + 1) * 512].bitcast(F32R),
                    start=(cc == 0 and t == 0),
                    stop=(cc == TPB - 1 and t == SB - 1),
                )
        if cc == TPB - 1:
            for half in range(2):
                nc.vector.tensor_copy(
                    out=ctx_raw[b * RP : b * RP + 1, half * 512 : (half + 1) * 512],
                    in_=cxacc[half],
                )
        del st[c]["scp"], st[c]["kchunk"]

    # prefetch the first chunks and keep PE busy while Wa streams in
    # ---- phase Q pieces: q = query @ Wa (+ biases), laid out as [h, b] ----
    qT_sb = const.tile([P, D2T, BL], F32)  # query transposed: [d2, j, b]
    q_sb = const.tile([BL, H], F32)
    qbT_sb = const.tile([P, HT, BL], F32)  # [h_in_tile, hi, b] tanh bias

    def emit_qT():
        for j in range(D2T):
            tp = tp_ps.tile([P, CHUNK], F32, tag="tp", name=f"qTp{j}")
            nc.tensor.transpose(
                tp[:, :BL], query_sb[:, j * P : (j + 1) * P], identity[:BL, :BL]
            )
            nc.vector.tensor_copy(
                out=qT_sb[:, j, :].bitcast(F32R), in_=tp[:, :BL].bitcast(F32R)
            )

    wa_tiles = {}

    def emit_wa_dmas(half, after_dma):
        sl = slice(half * 512, (half + 1) * 512)
        for j in range(D2T):
            wa_t = wa_pool.tile([P, 512], F32, tag="wa", name=f"wa{half}_{j}")
            dma = nc.sync.dma_start(
                out=wa_t.bitcast(F32R),
                in_=wa_d[j * P : (j + 1) * P, sl].bitcast(F32R),
            )
            if half == 0 and j == 0 and after_dma is not None:
                add_dep_helper(
                    dma.ins, after_dma.ins, reason="wa after ua preload"
                )
            wa_tiles[(half, j)] = wa_t

    def emit_q_mms(half):
        # q for h columns [half*512, (half+1)*512)
        sl = slice(half * 512, (half + 1) * 512)
        q_ps = cx_ps.tile([BL, 512], F32, tag="cx", name=f"q_ps{half}")
        for j in range(D2T):
            nc.tensor.matmul(
                q_ps,
                qT_sb[:, j, :].bitcast(F32R),
                wa_tiles[(half, j)].bitcast(F32R),
                start=(j == 0),
                stop=(j == D2T - 1),
            )
        nc.vector.tensor_add(q_sb[:, sl], q_ps, wab_bc[:, sl])
        nc.vector.tensor_add(q_sb[:, sl], q_sb[:, sl], uab_bc[:, sl])
        for hi in range(half * (HT // 2), (half + 1) * (HT // 2)):
            tp = tp_ps.tile([P, CHUNK], F32, tag="tp", name=f"qbTp{hi}")
            nc.tensor.transpose(
                tp[:, :BL], q_sb[:, hi * P : (hi + 1) * P], identity[:BL, :BL]
            )
            nc.vector.tensor_copy(out=qbT_sb[:, hi, :], in_=tp[:, :BL])

    # ---- startup orchestration ----
    # DMA chain: keys chunk 0 -> Ua -> {keys chunk 1, Wa}.  PE order: T(0),
    # qT, GEMM c0 h0-1, qMM-half0, GEMM h2-3, qMM-half1, GEMM h4-7 — the
    # splits respect the 2 GEMM psum slots (drained by tanh, which needs q).
    dma0 = emit_load(0)
    emit_misc_consts()
    ua_last = emit_ua_load(after_dma=dma0)
    k1_dma = emit_load(1)
    add_dep_helper(k1_dma.ins, ua_last.ins, reason="keys 1 after ua")
    emit_transpose(0)
    emit_qT()
    gp0 = emit_gemm(0, 0)
    gp1 = emit_gemm(0, 1)
    emit_wa_dmas(0, after_dma=ua_last)
    emit_wa_dmas(1, after_dma=None)
    emit_q_mms(0)
    emit_tanh(0, 0, gp0)
    emit_tanh(0, 1, gp1)
    emit_gemm_tanh(0, qbT_sb, hs=[2, 3])
    emit_q_mms(1)
    emit_gemm_tanh(0, qbT_sb, hs=[4, 5, 6, 7])

    # ---- main pass over keys (1-chunk-lag software pipeline) ----
    scores_sb = const.tile([P, S], F32)
    ctx_raw = const.tile([P, H], F32)  # unnormalized context rows
    w_sb = const.tile([P, S], F32)
    z_sb = const.tile([P, 1], F32)
    rz_sb = const.tile([P, 1], F32)
    ctx_sb = const.tile([P, H], F32)

    def emit_batch_tail(b):
        # softmax normalization + outputs for one finished batch; overlaps
        # the next batch's chunks (only the last batch's tail is serial)
        row = slice(b * RP, b * RP + 1)
        nc.scalar.activation(
            out=w_sb[row, :], in_=scores_sb[row, :], func=EXP, accum_out=z_sb[row, :]
        )
        nc.vector.reciprocal(out=rz_sb[row, :], in_=z_sb[row, :])
        nc.vector.tensor_scalar_mul(w_sb[row, :], w_sb[row, :], rz_sb[row, :])
        nc.sync.dma_start(out=wout_d[b : b + 1, :], in_=w_sb[row, :])
        nc.vector.tensor_scalar_mul(ctx_sb[row, :], ctx_raw[row, :], rz_sb[row, :])
        nc.sync.dma_start(out=ctx_d[b : b + 1, :], in_=ctx_sb[row, :])

    cxacc_ref = [None]
    for c in range(NCHUNK):
        # chunk c-1's exp -> ewT -> context ride on PE ahead of GEMM c,
        # so the ACT exp latency is hidden under the previous GEMM.
        if c > 0:
            emit_ctx(c - 1, cxacc_ref, ctx_raw)
            if c % TPB == 0:
                emit_batch_tail(c // TPB - 1)
            emit_gemm_tanh(c, qbT_sb)  # c == 0 was emitted during startup
        if c + 1 < NCHUNK:
            if c + 2 < NCHUNK:
                emit_load(c + 2)
            emit_transpose(c + 1)
        emit_scores(c, scores_sb)
    emit_ctx(NCHUNK - 1, cxacc_ref, ctx_raw)
    emit_batch_tail(BL - 1)


_CACHED_NC = None


def _get_nc():
    global _CACHED_NC
    if _CACHED_NC is None:
        _CACHED_NC = build_program()
    return _CACHED_NC


def make_in_maps(query, keys, Wa_w, Wa_b, Ua_w, Ua_b, Va_w):
    """Shard full inputs into 8 per-core input maps (data parallel over B)."""
    query = np.ascontiguousarray(np.asarray(query, dtype=np.float32)).reshape(B, H * L)
    keys = np.ascontiguousarray(np.asarray(keys, dtype=np.float32))
    shared = {
        "wa": np.ascontiguousarray(np.asarray(Wa_w, dtype=np.float32)),
        "wab": np.ascontiguousarray(np.asarray(Wa_b, dtype=np.float32)),
        "ua": np.ascontiguousarray(np.asarray(Ua_w, dtype=np.float32)),
        "uab": np.ascontiguousarray(np.asarray(Ua_b, dtype=np.float32)),
        "va": np.ascontiguousarray(np.asarray(Va_w, dtype=np.float32)),
    }
    in_maps = []
    for c in range(NCORES):
        sl = slice(c * BL, (c + 1) * BL)
        in_maps.append(
            {
                "query": np.ascontiguousarray(query[sl]),
                "keys": np.ascontiguousarray(keys[sl]),
                **shared,
            }
        )
    return in_maps


def assemble_outputs(results):
    ctxs = [np.asarray(r["context"]) for r in results]
    ws = [np.asarray(r["weights"]) for r in results]
    context = np.concatenate(ctxs, axis=0).reshape(B, 1, H)
    weights = np.concatenate(ws, axis=0).reshape(B, 1, S)
    return context.astype(np.float32), weights.astype(np.float32)


def run_on_hw(in_maps, trace=False, tmpdir=None):
    from concourse.bass_utils import run_bass_kernel_spmd

    nc = _get_nc()
    return run_bass_kernel_spmd(
        nc, in_maps, list(range(NCORES)), trace=trace, tmpdir=tmpdir
    )


def kernel(query, keys, Wa_w, Wa_b, Ua_w, Ua_b, Va_w, Va_b=None):
    # Va_b is mathematically irrelevant: softmax(s + c) == softmax(s), and
    # both outputs depend on the scores only through the softmax.
    in_maps = make_in_maps(query, keys, Wa_w, Wa_b, Ua_w, Ua_b, Va_w)
    res = run_on_hw(in_maps, trace=False)
    return assemble_outputs(res.results)


if __name__ == "__main__":
    nc = build_program()
    print("program built ok")
